# revision 26
# baseline (speedup 1.0000x reference)
"""Trainium2 Bass kernel for single-step LSTM decoder (batch 65536).

Contract: kernel(**inputs) takes FULL unsharded inputs (numpy, fp32) and
returns the FULL output tuple (logits, h, c) matching the reference.

Strategy:
  - Data-parallel: batch 65536 sharded 8192/core across 8 NeuronCores.
  - Host prepares transposed layouts (h0^T, c0^T, x^T with ones row) so the
    device kernel needs zero on-chip transposes; weights replicated per core.
  - Device pipeline (per core, transposed layout [feature, batch]):
      z^T[gate_chunk m] = U[kchunk, m].T @ h0T[kchunk] + W'[0:6, m].T @ xT1
      gates: sigmoid (ScalarE, from PSUM), relu (VectorE)
      c^T = f*c0 + i*g   (VectorE, fp32 result)
      h^T = o*relu(c)    (VectorE, bf16)
      logits^T = Wd.T @ h^T + bd  (PE + ScalarE bias)
  - Host transposes outputs back.
"""

import os
import numpy as np
import ml_dtypes

BATCH = 65536
NCORES = 8
BS = BATCH // NCORES  # 8192 per-core batch shard
NL = 256  # LSTM hidden
FEAT = 5
BT = 512  # batch tile (free dim of matmuls; one PSUM bank per gate chunk)
NST = BS // BT  # 16 super-tiles per core

BF16 = ml_dtypes.bfloat16

_CACHE = {}


def _build_nc():
    import concourse.tile as tile
    from concourse import bacc, mybir
    from contextlib import ExitStack

    BF = mybir.dt.bfloat16
    F32 = mybir.dt.float32
    AF = mybir.ActivationFunctionType

    nc = bacc.Bacc("TRN2", target_bir_lowering=False, num_devices=NCORES)

    h0T = nc.dram_tensor("h0T", [NL, BS], BF, kind="ExternalInput")
    c0T = nc.dram_tensor("c0T", [NL, BS], BF, kind="ExternalInput")
    xT1 = nc.dram_tensor("xT1", [FEAT + 1, BS], BF, kind="ExternalInput")
    U = nc.dram_tensor("U", [NL, 4 * NL], BF, kind="ExternalInput")
    Wp = nc.dram_tensor("Wp", [FEAT + 1, 4 * NL], BF, kind="ExternalInput")
    Wd = nc.dram_tensor("Wd", [NL, FEAT], BF, kind="ExternalInput")
    bd = nc.dram_tensor("bd", [FEAT, 1], F32, kind="ExternalInput")

    hT = nc.dram_tensor("hT", [NL, BS], BF, kind="ExternalOutput")
    cT = nc.dram_tensor("cT", [NL, BS], F32, kind="ExternalOutput")
    logT = nc.dram_tensor("logT", [FEAT, BS], F32, kind="ExternalOutput")

    with ExitStack() as ctx:
        tc = ctx.enter_context(tile.TileContext(nc, pool_alloc_mode="queue"))
        singles = ctx.enter_context(tc.tile_pool(name="singles", bufs=1))
        loads = ctx.enter_context(tc.tile_pool(name="loads", bufs=6))
        gates = ctx.enter_context(tc.tile_pool(name="gates", bufs=3))
        outsp = ctx.enter_context(tc.tile_pool(name="outs", bufs=5))
        zpool = ctx.enter_context(tc.tile_pool(name="zp", bufs=7, space="PSUM"))
        lpool = ctx.enter_context(tc.tile_pool(name="lp", bufs=1, space="PSUM"))

        # --- weight / constant preload (once) ---
        # Order matters: the first matmul needs U and the first h0 chunk, so
        # those DMAs are issued first; everything else follows.
        U_sb = []
        for k in range(2):
            t = singles.tile([128, 4 * NL], BF, tag=f"U{k}")
            nc.sync.dma_start(out=t, in_=U[k * 128 : (k + 1) * 128, :])
            U_sb.append(t)

        # Batch groups: a small first group so the PE starts early, then
        # large load chunks for DMA efficiency.
        groups = [(0, BT), (BT, BT), (2 * BT, 2 * BT), (4 * BT, 4 * BT),
                  (8 * BT, 4 * BT), (12 * BT, 4 * BT)]
        assert sum(w for _, w in groups) == BS

        def load_group(gstart, gwidth):
            h0_big, c0_big = [], []
            for k in range(2):
                t = loads.tile([128, 4 * BT], BF, tag=f"h0_{k}")
                nc.sync.dma_start(
                    out=t[:, :gwidth],
                    in_=h0T[k * 128 : (k + 1) * 128, gstart : gstart + gwidth],
                )
                h0_big.append(t)
            for j in range(2):
                t = loads.tile([128, 4 * BT], BF, tag=f"c0_{j}")
                nc.sync.dma_start(
                    out=t[:, :gwidth],
                    in_=c0T[j * 128 : (j + 1) * 128, gstart : gstart + gwidth],
                )
                c0_big.append(t)
            return h0_big, c0_big

        group_bufs = [load_group(*groups[0])]
        issued = 1

        # x^T (with ones row) and W' replicated at partition offsets 0/32/64/96
        # so the four K=6 x-matmuls can run concurrently in distinct PE
        # row-groups via tile_position.
        Wp_sb = singles.tile([96 + FEAT + 1, 4 * NL], BF, tag="Wp")
        xT1_sb = singles.tile([96 + FEAT + 1, BS], BF, tag="x")
        for g in range(4):
            r = 32 * g
            nc.sync.dma_start(out=Wp_sb[r : r + FEAT + 1, :], in_=Wp[:, :])
            nc.sync.dma_start(out=xT1_sb[r : r + FEAT + 1, :], in_=xT1[:, :])
        Wd_sb = []
        for k in range(2):
            t = singles.tile([128, FEAT], BF, tag=f"Wd{k}")
            nc.sync.dma_start(out=t, in_=Wd[k * 128 : (k + 1) * 128, :])
            Wd_sb.append(t)
        bd_sb = singles.tile([FEAT, 1], F32, tag="bd")
        nc.sync.dma_start(out=bd_sb, in_=bd[:, :])
        logacc = singles.tile([FEAT, BS], F32, tag="logacc")

        def flush_dense(pend):
            # Dense logits matmuls for a PREVIOUS super-tile: h is long ready,
            # so these never stall the in-order PE stream.
            h_pair, ps = pend
            lg = lpool.tile([FEAT, BT], F32, tag="lg")
            nc.tensor.matmul(lg, lhsT=Wd_sb[0], rhs=h_pair[0], start=True, stop=False)
            nc.tensor.matmul(lg, lhsT=Wd_sb[1], rhs=h_pair[1], start=False, stop=True)
            nc.scalar.activation(
                logacc[:, ps * BT : (ps + 1) * BT], lg, AF.Identity, bias=bd_sb
            )

        pend_dense = None
        sts = [(gi, gstart, off) for gi, (gstart, gw) in enumerate(groups)
               for off in range(0, gw, BT)]
        for gi, gstart, off in sts:
            s = (gstart + off) // BT
            bsl = slice(s * BT, (s + 1) * BT)
            if off == 0:
                while issued < min(gi + 3, len(groups)):
                    group_bufs.append(load_group(*groups[issued]))
                    issued += 1
                h0_big, c0_big = group_bufs[gi]
            osl = slice(off, off + BT)
            h0_t = [t[:, osl] for t in h0_big]
            c0_t = [t[:, osl] for t in c0_big]

            if pend_dense is not None:
                flush_dense(pend_dense)
            h_pair = []
            for j in range(2):
                # gate order in z columns: i, f, c(g), o ; chunk j covers
                # features j*128:(j+1)*128 of each gate block.
                zps = []
                for g in range(4):
                    col = g * NL + j * 128
                    zp = zpool.tile([128, BT], F32, tag="z")
                    nc.tensor.matmul(
                        zp, lhsT=U_sb[0][:, col : col + 128], rhs=h0_t[0],
                        start=True, stop=False,
                    )
                    nc.tensor.matmul(
                        zp, lhsT=U_sb[1][:, col : col + 128], rhs=h0_t[1],
                        start=False, stop=False,
                    )
                    zps.append(zp)
                # Four K=6 x-matmuls packed into distinct 32-row PE groups —
                # they run concurrently (one matmul slot instead of four).
                for g in range(4):
                    col = g * NL + j * 128
                    r = 32 * g
                    nc.tensor.matmul(
                        zps[g],
                        lhsT=Wp_sb[r : r + FEAT + 1, col : col + 128],
                        rhs=xT1_sb[r : r + FEAT + 1, bsl],
                        start=False, stop=True,
                        tile_position=(r, 0),
                    )

                i_t = gates.tile([128, BT], BF, tag="i")
                nc.scalar.activation(i_t, zps[0], AF.Sigmoid)
                f_t = gates.tile([128, BT], BF, tag="f")
                nc.scalar.activation(f_t, zps[1], AF.Sigmoid)
                o_t = gates.tile([128, BT], BF, tag="o")
                nc.scalar.activation(o_t, zps[3], AF.Sigmoid)
                g_t = gates.tile([128, BT], BF, tag="g")
                nc.vector.tensor_scalar_max(g_t, zps[2], 0.0)

                t1 = gates.tile([128, BT], BF, tag="t1")
                nc.vector.tensor_mul(t1, f_t, c0_t[j])
                t2 = gates.tile([128, BT], BF, tag="t2")
                nc.vector.tensor_mul(t2, i_t, g_t)
                c_t = outsp.tile([128, BT], F32, tag="c")
                nc.vector.tensor_add(c_t, t1, t2)
                store_eng = nc.sync if j == 0 else nc.scalar
                store_eng.dma_start(out=cT[j * 128 : (j + 1) * 128, bsl], in_=c_t)

                rc = gates.tile([128, BT], BF, tag="rc")
                nc.vector.tensor_scalar_max(rc, c_t, 0.0)
                h_t = outsp.tile([128, BT], BF, tag="h")
                nc.vector.tensor_mul(h_t, o_t, rc)
                store_eng.dma_start(out=hT[j * 128 : (j + 1) * 128, bsl], in_=h_t)
                h_pair.append(h_t)
            pend_dense = (h_pair, s)
        flush_dense(pend_dense)
        nc.sync.dma_start(out=logT[:, :], in_=logacc)

    nc.compile()
    return nc


def _get_nc():
    if "nc" not in _CACHE:
        _CACHE["nc"] = _build_nc()
    return _CACHE["nc"]


def _prep_in_maps(seq_in, h0, c0, W, U, b, Wd, bd):
    seq_in = np.asarray(seq_in, dtype=np.float32)
    h0 = np.asarray(h0, dtype=np.float32)
    c0 = np.asarray(c0, dtype=np.float32)
    W = np.asarray(W, dtype=np.float32)
    U = np.asarray(U, dtype=np.float32)
    b = np.asarray(b, dtype=np.float32)
    Wd = np.asarray(Wd, dtype=np.float32)
    bd = np.asarray(bd, dtype=np.float32)

    x = seq_in[:, 0, :]  # [B, 5]
    xT1 = np.empty((FEAT + 1, BATCH), dtype=BF16)
    xT1[:FEAT] = x.T.astype(BF16)
    xT1[FEAT] = np.ones((BATCH,), dtype=BF16)
    h0T = np.ascontiguousarray(h0.T).astype(BF16)  # [256, B]
    c0T = np.ascontiguousarray(c0.T).astype(BF16)
    Wp = np.empty((FEAT + 1, 4 * NL), dtype=BF16)
    Wp[:FEAT] = W.astype(BF16)
    Wp[FEAT] = b.astype(BF16)
    U_b = U.astype(BF16)
    Wd_b = Wd.astype(BF16)
    bd_c = np.ascontiguousarray(bd.reshape(FEAT, 1))

    in_maps = []
    for c in range(NCORES):
        sl = slice(c * BS, (c + 1) * BS)
        in_maps.append(
            {
                "h0T": np.ascontiguousarray(h0T[:, sl]),
                "c0T": np.ascontiguousarray(c0T[:, sl]),
                "xT1": np.ascontiguousarray(xT1[:, sl]),
                "U": U_b,
                "Wp": Wp,
                "Wd": Wd_b,
                "bd": bd_c,
            }
        )
    return in_maps


def _install_ntff_hook():
    """Register the axon NTFF profile hook (dev/tracing only).

    The trimmed antenv package in this container lacks axon_hooks; recreate
    it and wire the ctypes-based hook from trn_agent_boot so trace=True works.
    """
    import sys
    import types

    try:
        from antenv.axon_hooks import get_axon_ntff_profile_hook  # noqa: F401

        return
    except ImportError:
        pass
    import antenv
    from trn_agent_boot.trn_boot import _ntff_profile_via_ctypes

    mod = types.ModuleType("antenv.axon_hooks")
    state = {}
    mod.set_axon_ntff_profile_hook = lambda h: state.__setitem__("hook", h)
    mod.get_axon_ntff_profile_hook = lambda: state.get("hook")
    sys.modules["antenv.axon_hooks"] = mod
    antenv.axon_hooks = mod
    mod.set_axon_ntff_profile_hook(
        _ntff_profile_via_ctypes("/opt/axon/libaxon_pjrt.so")
    )


def _run(in_maps, trace=False):
    from concourse.bass_utils import run_bass_kernel_spmd

    if trace:
        _install_ntff_hook()
    nc = _get_nc()
    res = run_bass_kernel_spmd(
        nc, in_maps, core_ids=list(range(NCORES)), trace=trace
    )
    return res


def _gather(results):
    logits = np.empty((BATCH, FEAT), dtype=np.float32)
    h = np.empty((BATCH, NL), dtype=np.float32)
    c = np.empty((BATCH, NL), dtype=np.float32)
    for ci, r in enumerate(results):
        sl = slice(ci * BS, (ci + 1) * BS)
        logits[sl] = r["logT"].T
        h[sl] = r["hT"].astype(np.float32).T
        c[sl] = r["cT"].T
    return logits, h, c


def kernel(seq_in, h0, c0, W, U, b, Wd, bd):
    in_maps = _prep_in_maps(seq_in, h0, c0, W, U, b, Wd, bd)
    try:
        res = _run(in_maps, trace=False)
    except Exception:
        # One retry: transient device states (e.g. a wedged core from a
        # previous process) usually clear on re-execution.
        res = _run(in_maps, trace=False)
    return _gather(res.results)


def kernel_traced(seq_in, h0, c0, W, U, b, Wd, bd):
    """Like kernel() but with NTFF tracing; returns (outputs, exec_time_ns, res)."""
    in_maps = _prep_in_maps(seq_in, h0, c0, W, U, b, Wd, bd)
    res = _run(in_maps, trace=True)
    return _gather(res.results), res.exec_time_ns, res


# revision 28
# speedup vs baseline: 1.1906x; 1.1906x over previous
"""Trainium2 Bass kernel for single-step LSTM decoder (batch 65536).

Contract: kernel(**inputs) takes FULL unsharded inputs (numpy, fp32) and
returns the FULL output tuple (logits, h, c) matching the reference.

Strategy:
  - Data-parallel: batch 65536 sharded 8192/core across 8 NeuronCores.
  - Host prepares transposed layouts (h0^T, c0^T, x^T with ones row) so the
    device kernel needs zero on-chip transposes; weights replicated per core.
  - Device pipeline (per core, transposed layout [feature, batch]):
      z^T[gate_chunk m] = U[kchunk, m].T @ h0T[kchunk] + W'[0:6, m].T @ xT1
      gates: sigmoid (ScalarE, from PSUM), relu (VectorE)
      c^T = f*c0 + i*g   (VectorE, fp32 result)
      h^T = o*relu(c)    (VectorE, bf16)
      logits^T = Wd.T @ h^T + bd  (PE + ScalarE bias)
  - Host transposes outputs back.
"""

import os
import numpy as np
import ml_dtypes

BATCH = 65536
NCORES = 8
BS = BATCH // NCORES  # 8192 per-core batch shard
NL = 256  # LSTM hidden
FEAT = 5
BT = 512  # batch tile (free dim of matmuls; one PSUM bank per gate chunk)
NST = BS // BT  # 16 super-tiles per core

BF16 = ml_dtypes.bfloat16

_CACHE = {}


def _build_nc():
    import concourse.tile as tile
    from concourse import bacc, mybir
    from contextlib import ExitStack

    BF = mybir.dt.bfloat16
    F32 = mybir.dt.float32
    AF = mybir.ActivationFunctionType

    nc = bacc.Bacc("TRN2", target_bir_lowering=False, num_devices=NCORES)

    h0T = nc.dram_tensor("h0T", [NL, BS], BF, kind="ExternalInput")
    c0T = nc.dram_tensor("c0T", [NL, BS], BF, kind="ExternalInput")
    xT1 = nc.dram_tensor("xT1", [FEAT + 1, BS], BF, kind="ExternalInput")
    U = nc.dram_tensor("U", [NL, 4 * NL], BF, kind="ExternalInput")
    Wp = nc.dram_tensor("Wp", [FEAT + 1, 4 * NL], BF, kind="ExternalInput")
    Wd = nc.dram_tensor("Wd", [NL, FEAT], BF, kind="ExternalInput")
    bd = nc.dram_tensor("bd", [FEAT, 1], F32, kind="ExternalInput")

    hT = nc.dram_tensor("hT", [NL, BS], BF, kind="ExternalOutput")
    cT = nc.dram_tensor("cT", [NL, BS], BF, kind="ExternalOutput")
    logT = nc.dram_tensor("logT", [FEAT, BS], F32, kind="ExternalOutput")

    with ExitStack() as ctx:
        tc = ctx.enter_context(tile.TileContext(nc))
        singles = ctx.enter_context(tc.tile_pool(name="singles", bufs=1))
        loads = ctx.enter_context(tc.tile_pool(name="loads", bufs=6))
        gates = ctx.enter_context(tc.tile_pool(name="gates", bufs=3))
        outsp = ctx.enter_context(tc.tile_pool(name="outs", bufs=5))
        zpool = ctx.enter_context(tc.tile_pool(name="zp", bufs=7, space="PSUM"))
        lpool = ctx.enter_context(tc.tile_pool(name="lp", bufs=1, space="PSUM"))

        # --- weight / constant preload (once) ---
        # Order matters: the first matmul needs U and the first h0 chunk, so
        # those DMAs are issued first; everything else follows.
        U_sb = []
        for k in range(2):
            t = singles.tile([128, 4 * NL], BF, tag=f"U{k}")
            nc.sync.dma_start(out=t, in_=U[k * 128 : (k + 1) * 128, :])
            U_sb.append(t)

        # Batch groups: a small first group so the PE starts early, then
        # large load chunks for DMA efficiency.
        groups = [(0, BT), (BT, BT), (2 * BT, 2 * BT), (4 * BT, 4 * BT),
                  (8 * BT, 4 * BT), (12 * BT, 4 * BT)]
        assert sum(w for _, w in groups) == BS

        def load_group(gstart, gwidth):
            h0_big, c0_big = [], []
            for k in range(2):
                t = loads.tile([128, 4 * BT], BF, tag=f"h0_{k}")
                nc.sync.dma_start(
                    out=t[:, :gwidth],
                    in_=h0T[k * 128 : (k + 1) * 128, gstart : gstart + gwidth],
                )
                h0_big.append(t)
            for j in range(2):
                t = loads.tile([128, 4 * BT], BF, tag=f"c0_{j}")
                nc.sync.dma_start(
                    out=t[:, :gwidth],
                    in_=c0T[j * 128 : (j + 1) * 128, gstart : gstart + gwidth],
                )
                c0_big.append(t)
            return h0_big, c0_big

        group_bufs = [load_group(*groups[0])]
        issued = 1

        # x^T (with ones row) and W' replicated at partition offsets 0/32/64/96
        # so the four K=6 x-matmuls can run concurrently in distinct PE
        # row-groups via tile_position.
        Wp_sb = singles.tile([96 + FEAT + 1, 4 * NL], BF, tag="Wp")
        xT1_sb = singles.tile([96 + FEAT + 1, BS], BF, tag="x")
        for g in range(4):
            r = 32 * g
            nc.sync.dma_start(out=Wp_sb[r : r + FEAT + 1, :], in_=Wp[:, :])
            nc.sync.dma_start(out=xT1_sb[r : r + FEAT + 1, :], in_=xT1[:, :])
        Wd_sb = []
        for k in range(2):
            t = singles.tile([128, FEAT], BF, tag=f"Wd{k}")
            nc.sync.dma_start(out=t, in_=Wd[k * 128 : (k + 1) * 128, :])
            Wd_sb.append(t)
        bd_sb = singles.tile([FEAT, 1], F32, tag="bd")
        nc.sync.dma_start(out=bd_sb, in_=bd[:, :])
        logacc = singles.tile([FEAT, BS], F32, tag="logacc")

        def flush_dense(pend):
            # Dense logits matmuls for a PREVIOUS super-tile: h is long ready,
            # so these never stall the in-order PE stream.
            h_pair, ps = pend
            lg = lpool.tile([FEAT, BT], F32, tag="lg")
            nc.tensor.matmul(lg, lhsT=Wd_sb[0], rhs=h_pair[0], start=True, stop=False)
            nc.tensor.matmul(lg, lhsT=Wd_sb[1], rhs=h_pair[1], start=False, stop=True)
            nc.scalar.activation(
                logacc[:, ps * BT : (ps + 1) * BT], lg, AF.Identity, bias=bd_sb
            )

        pend_dense = None
        sts = [(gi, gstart, off) for gi, (gstart, gw) in enumerate(groups)
               for off in range(0, gw, BT)]
        for gi, gstart, off in sts:
            s = (gstart + off) // BT
            bsl = slice(s * BT, (s + 1) * BT)
            if off == 0:
                while issued < min(gi + 3, len(groups)):
                    group_bufs.append(load_group(*groups[issued]))
                    issued += 1
                h0_big, c0_big = group_bufs[gi]
            osl = slice(off, off + BT)
            h0_t = [t[:, osl] for t in h0_big]
            c0_t = [t[:, osl] for t in c0_big]

            if pend_dense is not None:
                flush_dense(pend_dense)
            h_pair = []
            for j in range(2):
                # gate order in z columns: i, f, c(g), o ; chunk j covers
                # features j*128:(j+1)*128 of each gate block.
                zps = []
                for g in range(4):
                    col = g * NL + j * 128
                    zp = zpool.tile([128, BT], F32, tag="z")
                    nc.tensor.matmul(
                        zp, lhsT=U_sb[0][:, col : col + 128], rhs=h0_t[0],
                        start=True, stop=False,
                    )
                    nc.tensor.matmul(
                        zp, lhsT=U_sb[1][:, col : col + 128], rhs=h0_t[1],
                        start=False, stop=False,
                    )
                    zps.append(zp)
                # Four K=6 x-matmuls packed into distinct 32-row PE groups —
                # they run concurrently (one matmul slot instead of four).
                for g in range(4):
                    col = g * NL + j * 128
                    r = 32 * g
                    nc.tensor.matmul(
                        zps[g],
                        lhsT=Wp_sb[r : r + FEAT + 1, col : col + 128],
                        rhs=xT1_sb[r : r + FEAT + 1, bsl],
                        start=False, stop=True,
                        tile_position=(r, 0),
                    )

                i_t = gates.tile([128, BT], BF, tag="i")
                nc.scalar.activation(i_t, zps[0], AF.Sigmoid)
                f_t = gates.tile([128, BT], BF, tag="f")
                nc.scalar.activation(f_t, zps[1], AF.Sigmoid)
                o_t = gates.tile([128, BT], BF, tag="o")
                nc.scalar.activation(o_t, zps[3], AF.Sigmoid)
                g_t = gates.tile([128, BT], BF, tag="g")
                nc.vector.tensor_scalar_max(g_t, zps[2], 0.0)

                t1 = gates.tile([128, BT], BF, tag="t1")
                nc.vector.tensor_mul(t1, f_t, c0_t[j])
                t2 = gates.tile([128, BT], BF, tag="t2")
                nc.vector.tensor_mul(t2, i_t, g_t)
                c_t = outsp.tile([128, BT], BF, tag="c")
                nc.vector.tensor_add(c_t, t1, t2)
                store_eng = nc.sync if j == 0 else nc.scalar
                store_eng.dma_start(out=cT[j * 128 : (j + 1) * 128, bsl], in_=c_t)

                rc = gates.tile([128, BT], BF, tag="rc")
                nc.vector.tensor_scalar_max(rc, c_t, 0.0)
                h_t = outsp.tile([128, BT], BF, tag="h")
                nc.vector.tensor_mul(h_t, o_t, rc)
                store_eng.dma_start(out=hT[j * 128 : (j + 1) * 128, bsl], in_=h_t)
                h_pair.append(h_t)
            pend_dense = (h_pair, s)
        flush_dense(pend_dense)
        nc.sync.dma_start(out=logT[:, :], in_=logacc)

    nc.compile()
    return nc


def _get_nc():
    if "nc" not in _CACHE:
        _CACHE["nc"] = _build_nc()
    return _CACHE["nc"]


def _prep_in_maps(seq_in, h0, c0, W, U, b, Wd, bd):
    seq_in = np.asarray(seq_in, dtype=np.float32)
    h0 = np.asarray(h0, dtype=np.float32)
    c0 = np.asarray(c0, dtype=np.float32)
    W = np.asarray(W, dtype=np.float32)
    U = np.asarray(U, dtype=np.float32)
    b = np.asarray(b, dtype=np.float32)
    Wd = np.asarray(Wd, dtype=np.float32)
    bd = np.asarray(bd, dtype=np.float32)

    x = seq_in[:, 0, :]  # [B, 5]
    xT1 = np.empty((FEAT + 1, BATCH), dtype=BF16)
    xT1[:FEAT] = x.T.astype(BF16)
    xT1[FEAT] = np.ones((BATCH,), dtype=BF16)
    h0T = np.ascontiguousarray(h0.T).astype(BF16)  # [256, B]
    c0T = np.ascontiguousarray(c0.T).astype(BF16)
    Wp = np.empty((FEAT + 1, 4 * NL), dtype=BF16)
    Wp[:FEAT] = W.astype(BF16)
    Wp[FEAT] = b.astype(BF16)
    U_b = U.astype(BF16)
    Wd_b = Wd.astype(BF16)
    bd_c = np.ascontiguousarray(bd.reshape(FEAT, 1))

    in_maps = []
    for c in range(NCORES):
        sl = slice(c * BS, (c + 1) * BS)
        in_maps.append(
            {
                "h0T": np.ascontiguousarray(h0T[:, sl]),
                "c0T": np.ascontiguousarray(c0T[:, sl]),
                "xT1": np.ascontiguousarray(xT1[:, sl]),
                "U": U_b,
                "Wp": Wp,
                "Wd": Wd_b,
                "bd": bd_c,
            }
        )
    return in_maps


def _install_ntff_hook():
    """Register the axon NTFF profile hook (dev/tracing only).

    The trimmed antenv package in this container lacks axon_hooks; recreate
    it and wire the ctypes-based hook from trn_agent_boot so trace=True works.
    """
    import sys
    import types

    try:
        from antenv.axon_hooks import get_axon_ntff_profile_hook  # noqa: F401

        return
    except ImportError:
        pass
    import antenv
    from trn_agent_boot.trn_boot import _ntff_profile_via_ctypes

    mod = types.ModuleType("antenv.axon_hooks")
    state = {}
    mod.set_axon_ntff_profile_hook = lambda h: state.__setitem__("hook", h)
    mod.get_axon_ntff_profile_hook = lambda: state.get("hook")
    sys.modules["antenv.axon_hooks"] = mod
    antenv.axon_hooks = mod
    mod.set_axon_ntff_profile_hook(
        _ntff_profile_via_ctypes("/opt/axon/libaxon_pjrt.so")
    )


def _run(in_maps, trace=False):
    from concourse.bass_utils import run_bass_kernel_spmd

    if trace:
        _install_ntff_hook()
    nc = _get_nc()
    res = run_bass_kernel_spmd(
        nc, in_maps, core_ids=list(range(NCORES)), trace=trace
    )
    return res


def _gather(results):
    logits = np.empty((BATCH, FEAT), dtype=np.float32)
    h = np.empty((BATCH, NL), dtype=np.float32)
    c = np.empty((BATCH, NL), dtype=np.float32)
    for ci, r in enumerate(results):
        sl = slice(ci * BS, (ci + 1) * BS)
        logits[sl] = r["logT"].T
        h[sl] = r["hT"].astype(np.float32).T
        c[sl] = r["cT"].astype(np.float32).T
    return logits, h, c


def kernel(seq_in, h0, c0, W, U, b, Wd, bd):
    in_maps = _prep_in_maps(seq_in, h0, c0, W, U, b, Wd, bd)
    try:
        res = _run(in_maps, trace=False)
    except Exception:
        # One retry: transient device states (e.g. a wedged core from a
        # previous process) usually clear on re-execution.
        res = _run(in_maps, trace=False)
    return _gather(res.results)


def kernel_traced(seq_in, h0, c0, W, U, b, Wd, bd):
    """Like kernel() but with NTFF tracing; returns (outputs, exec_time_ns, res)."""
    in_maps = _prep_in_maps(seq_in, h0, c0, W, U, b, Wd, bd)
    res = _run(in_maps, trace=True)
    return _gather(res.results), res.exec_time_ns, res
